# revision 23
# baseline (speedup 1.0000x reference)
"""Trainium2 Bass kernel for nn_Cca3 channel cross-attention.

Reference computation (per pair b of 8):
  x_s, x_t : [128, N] (N = 128*128 spatial), C = 128 channels
  q/k/v = 1x1 conv projections (w @ x + b) of both streams
  S1 = q_t @ k_s^T  (contract over N) -> a_st = rowsoftmax(S1)
  S2 = q_s @ k_t^T                    -> a_ts = rowsoftmax(S2)
  att = rowsoftmax(a_st @ a_ts^T)
  out_s = x_s + att @ v_s ; out_t = x_t + att @ v_t

Sharding: data-parallel, one (x_s[i], x_t[i]) pair per NeuronCore (8 cores).

Device strategy (per core), fp16 operands / fp32 PSUM accumulation:
  - DMA economics: every dma_start costs ~0.7 us of issue time on the Sync
    engine and one queue runs at ~22 GB/s, so input is 16x2 slabs of 128 KB
    (512 cols) issued first, weights packed into two small tensors, and
    output rides 256 KB DMAs with a tapered tail.
  - Warmup burst: back-to-back 512-wide matmuls on a zero scratch flip the
    PE HAM clock gate to 2.4 GHz while the input DMA pipe fills (otherwise
    the short phase-1 matmuls never fill an activity window and the whole
    kernel runs at 1.2 GHz).
  - Scores via the Gram matrix: with projections P = w X (sans bias),
      S2_raw = qw (Xs Xt^T) kw^T = qw G kw^T,   S1_raw = qw G^T kw^T.
    Phase 1 only accumulates G [128,128]: per 128-col chunk, PE-transpose
    xs/xt chunks (fp16) into a ring; G matmuls trail by one 4-chunk batch
    (transposes batched so the identity stationary loads once per batch).
    Transposes of 2 chunks share one PSUM tile (PSUM is bank-granular);
    ring copies are whole-tile [128,512] DVE/ACT ops, alternating.
  - Bias corrections are rank-1: S1 += outer(qb, K0s + N kb) + outer(Q0t, kb)
    with the row vectors (projection column sums = w @ colsum(x)) computed on
    host from the same fp16 x the device sees, fed as tiny fp16 inputs.
  - Epilogue (all [128,128], fp16 operands): G -> S1/S2 via two matmuls
    each, softmax chains on DVE+ACT (exp with accum_out gives the row sum
    free), att composition via PE transposes + one 128^3 matmul.
  - Phase 2 folds v-projection, attention apply, and residual into ONE
    stationary weight:  y = (att vw + I) x + (att vb) (x) 1
    -> per 1024-col tile: two fp16 matmuls (constant stationary W'^T),
    two biased copies PSUM->SBUF fp16 (ACT/DVE alternating), one 256 KB
    output DMA; the last tiles shrink to keep the drain tail short.
"""

from contextlib import ExitStack

import numpy as np

C = 128
N_FULL = 16384
SLAB = 512  # input DMA slab width (fp16 -> 128 KB per slab)
F2 = 512  # phase-2 matmul/copy width (one PSUM bank)
TSLOT = 256  # ring slot: [xTs(128) | xTt(128)], fp16
TBUFS = 12  # ring depth (even, so chunk pairs land in adjacent slots)
TBATCH = 4  # transpose batch (identity stationary reloads once per batch)
NWARM = 16  # HAM warmup matmuls


def build_nc(n=N_FULL):
    import concourse.bacc as bacc
    import concourse.tile as tile
    from concourse import mybir
    from concourse.masks import make_identity

    f32 = mybir.dt.float32
    f16 = mybir.dt.float16
    AF = mybir.ActivationFunctionType
    AX = mybir.AxisListType

    slab = min(SLAB, n)
    nslabs = n // slab
    nchunks = n // C
    nbatches = nchunks // TBATCH
    assert TBUFS % 2 == 0 and TBUFS >= 2 * TBATCH

    nc = bacc.Bacc("TRN2", target_bir_lowering=False, debug=False)

    def din(name, shape, dt=f16):
        return nc.dram_tensor(name, shape, dt, kind="ExternalInput").ap()

    def dout(name, shape, dt=f16):
        return nc.dram_tensor(name, shape, dt, kind="ExternalOutput").ap()

    xs_d = din("xs", [C, n])
    xt_d = din("xt", [C, n])
    # packed weights: [qwT | kwT | vw | vb_col]
    wpack_d = din("wpack", [C, 3 * C + 1])
    # packed rows: [qb | kb | cks | ckt | q0s | q0t]
    rpack_d = din("rpack", [1, 6 * C])
    ys_d = dout("ys", [C, n])
    yt_d = dout("yt", [C, n])

    with tile.TileContext(nc) as tc, ExitStack() as ctx:
        singles = ctx.enter_context(tc.tile_pool(name="singles", bufs=1))

        # ---- persistent SBUF ----
        xs_sb = singles.tile([C, n], f16, tag="xs")
        xt_sb = singles.tile([C, n], f16, tag="xt")
        wpack_sb = singles.tile([C, 3 * C + 1], f16, tag="wpack")
        rpack_sb = singles.tile([1, 6 * C], f16, tag="rpack")
        ident16 = singles.tile([C, C], f16, tag="ident16")
        tring = singles.tile([C, TBUFS * TSLOT], f16, tag="tring")
        scratch = singles.tile([C, 512], f16, tag="scratch")
        warm_sb = singles.tile([1, 2], f32, tag="warm")

        qwT_sb = wpack_sb[:, 0:C]
        kwT_sb = wpack_sb[:, C : 2 * C]
        vw_sb = wpack_sb[:, 2 * C : 3 * C]
        vb_sb = wpack_sb[:, 3 * C : 3 * C + 1]
        qb_sb = rpack_sb[0:1, 0:C]
        kb_sb = rpack_sb[0:1, C : 2 * C]
        cks_sb = rpack_sb[0:1, 2 * C : 3 * C]
        ckt_sb = rpack_sb[0:1, 3 * C : 4 * C]
        q0s_sb = rpack_sb[0:1, 4 * C : 5 * C]
        q0t_sb = rpack_sb[0:1, 5 * C : 6 * C]

        # ---- input slabs first: Sync issue slots are the scarce resource
        for k in range(nslabs):
            sl = slice(k * slab, (k + 1) * slab)
            nc.sync.dma_start(out=xs_sb[:, sl], in_=xs_d[:, sl])
            nc.sync.dma_start(out=xt_sb[:, sl], in_=xt_d[:, sl])
            if k == 1:
                nc.sync.dma_start(out=wpack_sb, in_=wpack_d)
                nc.sync.dma_start(out=rpack_sb, in_=rpack_d)

        # HAM warmup: long back-to-back matmuls while the input pipe fills
        nc.vector.memset(scratch, 0.0)
        make_identity(nc, ident16)
        with tc.tile_pool(name="warmps", bufs=2, space="PSUM") as warm_ps:
            for _ in range(NWARM):
                wp = warm_ps.tile([C, 512], f32, tag="wp")
                nc.tensor.matmul(wp, lhsT=scratch[:, 0:C], rhs=scratch,
                                 start=True, stop=True)
        # warm the ACT exp table early (overlaps input DMA)
        nc.vector.memset(warm_sb, 0.0)
        nc.scalar.activation(out=warm_sb, in_=warm_sb, func=AF.Exp)

        # =========================== phase 1 ===========================
        smalls = ctx.enter_context(tc.tile_pool(name="smalls", bufs=1))
        g16_sb = smalls.tile([C, C], f16, tag="g16")
        gt16_sb = smalls.tile([C, C], f16, tag="gt16")
        m1_sb = smalls.tile([C, C], f16, tag="m1")
        m2_sb = smalls.tile([C, C], f16, tag="m2")
        ast_sb = smalls.tile([C, C], f16, tag="ast")
        ats_sb = smalls.tile([C, C], f16, tag="ats")
        att_sb = smalls.tile([C, C], f16, tag="att")
        astT_sb = smalls.tile([C, C], f16, tag="astT")
        atsT_sb = smalls.tile([C, C], f16, tag="atsT")
        attT_sb = smalls.tile([C, C], f16, tag="attT")
        wt_sb = smalls.tile([C, C], f16, tag="wt")  # (E_m vw + diag(s_m))^T
        identS_sb = smalls.tile([C, C], f16, tag="identS")  # diag(s_m)
        ceff2_sb = smalls.tile([C, 1], f32, tag="ceff2")  # (E_m vb) * rinv_m
        rinvm_sb = smalls.tile([C, 1], f32, tag="rinvm")  # 1 / s_m
        biasm_sb = smalls.tile([C, 1], f32, tag="biasm")

        with tc.tile_pool(name="gps", bufs=1, space="PSUM") as g_ps_pool:
            G = g_ps_pool.tile([C, C], f32, tag="G")

            with tc.tile_pool(name="trps", bufs=4, space="PSUM") as tr_ps:

                def emit_tr_pair(p):
                    # chunks 2p, 2p+1 -> one PSUM tile, one ring copy
                    i0 = 2 * p
                    st = (i0 % TBUFS) * TSLOT
                    psT = tr_ps.tile([C, 4 * C], f16, tag="psT")
                    for u in range(2):
                        sl = slice((i0 + u) * C, (i0 + u + 1) * C)
                        nc.tensor.transpose(psT[:, 2 * u * C : (2 * u + 1) * C],
                                            xs_sb[:, sl], ident16)
                        nc.tensor.transpose(
                            psT[:, (2 * u + 1) * C : (2 * u + 2) * C],
                            xt_sb[:, sl], ident16)
                    if p % 16 < 9:
                        nc.vector.tensor_copy(tring[:, st : st + 4 * C], psT)
                    else:
                        nc.scalar.copy(tring[:, st : st + 4 * C], psT)

                def emit_g_batch(b):
                    for j in range(b * TBATCH, (b + 1) * TBATCH):
                        st = (j % TBUFS) * TSLOT
                        nc.tensor.matmul(G, lhsT=tring[:, st : st + C],
                                         rhs=tring[:, st + C : st + 2 * C],
                                         start=(j == 0),
                                         stop=(j == nchunks - 1))

                pairs_per_batch = TBATCH // 2
                for b in range(nbatches + 1):
                    if b < nbatches:
                        for p in range(b * pairs_per_batch,
                                       (b + 1) * pairs_per_batch):
                            emit_tr_pair(p)
                    if b >= 1:
                        emit_g_batch(b - 1)

            nc.vector.tensor_copy(g16_sb, G)

        # ---- epilogue: G -> S1/S2 -> att -> W' (all fp16) ----
        def rowsoftmax(src, dst, tg):
            nmx = smalls.tile([C, 1], f32, tag=tg + "nmx")
            ssum = smalls.tile([C, 1], f32, tag=tg + "ssum")
            rinv = smalls.tile([C, 1], f32, tag=tg + "rinv")
            nc.vector.reduce_max(nmx, src, axis=AX.X, negate=True)
            nc.scalar.activation(out=dst, in_=src, func=AF.Exp,
                                 bias=nmx, scale=1.0, accum_out=ssum)
            nc.vector.reciprocal(rinv, ssum)
            nc.vector.tensor_scalar_mul(dst, dst, rinv)

        with tc.tile_pool(name="eps", bufs=2, space="PSUM") as e_ps, \
             tc.tile_pool(name="sps", bufs=1, space="PSUM") as s_ps, \
             tc.tile_pool(name="fillps", bufs=1, space="PSUM") as fill_ps:
            # independent scratch matmuls keep the PE activity monitor happy
            # through the epilogue's serial chain (else the clock gate drops
            # back to 1.2 GHz and phase 2 starts cold)
            fp = fill_ps.tile([C, 512], f32, tag="fp")

            def pe_fill(k=1):
                for _ in range(k):
                    nc.tensor.matmul(fp, lhsT=scratch[:, 0:C], rhs=scratch,
                                     start=True, stop=True)

            # S2 branch first (no dependence on the G transpose)
            m2_ps = e_ps.tile([C, C], f32, tag="e")
            nc.tensor.matmul(m2_ps, lhsT=g16_sb, rhs=qwT_sb,
                             start=True, stop=True)
            gt_ps = e_ps.tile([C, C], f16, tag="t")
            nc.tensor.transpose(gt_ps, g16_sb, ident16)
            nc.vector.tensor_copy(m2_sb, m2_ps)
            nc.scalar.copy(gt16_sb, gt_ps)
            pe_fill(2)
            S2 = s_ps.tile([C, C], f32, tag="S2")
            S1 = s_ps.tile([C, C], f32, tag="S1")
            nc.tensor.matmul(S2, lhsT=m2_sb, rhs=kwT_sb,
                             start=True, stop=True)
            nc.tensor.matmul(S2, lhsT=qb_sb, rhs=ckt_sb, start=False,
                             stop=False, skip_group_check=True)
            nc.tensor.matmul(S2, lhsT=q0s_sb, rhs=kb_sb, start=False,
                             stop=True, skip_group_check=True)
            m1_ps = e_ps.tile([C, C], f32, tag="e")
            nc.tensor.matmul(m1_ps, lhsT=gt16_sb, rhs=qwT_sb,
                             start=True, stop=True)
            nc.vector.tensor_copy(m1_sb, m1_ps)
            pe_fill(2)
            nc.tensor.matmul(S1, lhsT=m1_sb, rhs=kwT_sb,
                             start=True, stop=True)
            nc.tensor.matmul(S1, lhsT=qb_sb, rhs=cks_sb, start=False,
                             stop=False, skip_group_check=True)
            nc.tensor.matmul(S1, lhsT=q0t_sb, rhs=kb_sb, start=False,
                             stop=True, skip_group_check=True)

            pe_fill(3)
            # a_ts: full rowsoftmax (its row scale changes the att logits)
            rowsoftmax(S2, ats_sb, "s2")
            # a_st: UNnormalized exp; its 1/sum folds into the att-exp scale
            nmx1 = smalls.tile([C, 1], f32, tag="nmx1")
            ssum1 = smalls.tile([C, 1], f32, tag="ssum1")
            rinv1 = smalls.tile([C, 1], f32, tag="rinv1")
            nc.vector.reduce_max(nmx1, S1, axis=AX.X, negate=True)
            nc.scalar.activation(out=ast_sb, in_=S1, func=AF.Exp,
                                 bias=nmx1, scale=1.0, accum_out=ssum1)
            nc.vector.reciprocal(rinv1, ssum1)
            pe_fill(3)
            t2 = e_ps.tile([C, C], f16, tag="t")
            nc.tensor.transpose(t2, ats_sb, ident16)
            nc.scalar.copy(atsT_sb, t2)
            t1 = e_ps.tile([C, C], f16, tag="t")
            nc.tensor.transpose(t1, ast_sb, ident16)
            nc.vector.tensor_copy(astT_sb, t1)
            m_ps = e_ps.tile([C, C], f32, tag="e")
            nc.tensor.matmul(m_ps, lhsT=astT_sb, rhs=atsT_sb,
                             start=True, stop=True)
            pe_fill(4)
            # att = softmax(rinv1 * Z_u) row-wise, kept UNnormalized:
            # E_m = exp(Z_u*rinv1 - max*rinv1); 1/s_m folds into phase 2
            nmz = smalls.tile([C, 1], f32, tag="nmz")
            ssumm = smalls.tile([C, 1], f32, tag="ssumm")
            nc.vector.reduce_max(nmz, m_ps, axis=AX.X, negate=True)
            nc.vector.tensor_mul(biasm_sb, nmz, rinv1)
            nc.scalar.activation(out=att_sb, in_=m_ps, func=AF.Exp,
                                 bias=biasm_sb, scale=rinv1,
                                 accum_out=ssumm)
            nc.vector.reciprocal(rinvm_sb, ssumm)
            nc.vector.tensor_scalar_mul(identS_sb, ident16, ssumm)
            pe_fill(3)
            t3 = e_ps.tile([C, C], f16, tag="t")
            nc.tensor.transpose(t3, att_sb, ident16)
            nc.vector.tensor_copy(attT_sb, t3)
            # W''^T = (E_m vw)^T + diag(s_m); y = (W'' x + E_m vb) * rinv_m
            wt_ps = e_ps.tile([C, C], f32, tag="e")
            nc.tensor.matmul(wt_ps, lhsT=vw_sb, rhs=attT_sb,
                             start=True, stop=True)
            nc.vector.tensor_add(wt_sb, wt_ps, identS_sb)
            ce_ps = s_ps.tile([C, 1], f32, tag="ce")
            nc.tensor.matmul(ce_ps, lhsT=attT_sb, rhs=vb_sb,
                             start=True, stop=True)
            nc.vector.tensor_mul(ceff2_sb, ce_ps, rinvm_sb)

        # =========================== phase 2 ===========================
        # y = W' x + ceff (x) 1 ; W'^T constant stationary for all chunks.
        # One y tile = `parts` x F2 columns, one output DMA per tile;
        # tail tiles shrink so the final queue drain is short.
        # per stream: 14 x 1024-col tiles, 2 x 512, 2 x 256 (tapered tail so
        # the final per-queue drain is ~3 us instead of ~12)
        spans = [(j * 1024, 1024) for j in range(14)]
        spans += [(14336, 512), (14848, 512), (15360, 512),
                  (15872, 256), (16128, 256)]
        assert sum(w for _, w in spans) == n
        tiles = []  # (x_sb, y_d, col_start, width)
        for col, w in spans:
            tiles.append((xs_sb, ys_d, col, w))
            tiles.append((xt_sb, yt_d, col, w))

        with tc.tile_pool(name="ops", bufs=6, space="PSUM") as o_ps_pool, \
             tc.tile_pool(name="ysb", bufs=12) as y_sb_pool:
            ci = 0
            for idx, (x_sb, y_d, col, width) in enumerate(tiles):
                y_sb = y_sb_pool.tile([C, 2 * F2], f16, tag="y")
                for h in range(0, width, F2):
                    w = min(F2, width - h)
                    sl = slice(col + h, col + h + w)
                    o_ps = o_ps_pool.tile([C, F2], f32, tag="o")
                    nc.tensor.matmul(o_ps[:, 0:w], lhsT=wt_sb,
                                     rhs=x_sb[:, sl], start=True, stop=True)
                    dst = y_sb[:, h : h + w]
                    ci += 1
                    if ci % 2 == 0:
                        nc.vector.tensor_scalar(
                            dst, o_ps[:, 0:w], rinvm_sb, ceff2_sb,
                            mybir.AluOpType.mult, mybir.AluOpType.add)
                    else:
                        nc.scalar.activation(out=dst, in_=o_ps[:, 0:w],
                                             func=AF.Identity,
                                             bias=ceff2_sb, scale=rinvm_sb)
                # two issue engines: Sync and GpSimd (SWDGE) share the
                # per-dma ~0.7us descriptor-generation cost
                eng = nc.gpsimd if idx % 5 < 2 else nc.sync
                eng.dma_start(out=y_d[:, col : col + width],
                              in_=y_sb[:, 0:width])

    nc.compile()
    return nc


def prep_core_inputs(x, qw, qb, kw, kb, vw, vb, n=N_FULL):
    """Build the 8 per-core input maps from full inputs."""
    f32, f16 = np.float32, np.float16
    qw = qw.astype(f32)
    kw = kw.astype(f32)
    wpack = np.concatenate(
        [qw.T, kw.T, vw.astype(f32), vb.reshape(C, 1)], axis=1).astype(f16)
    wpack = np.ascontiguousarray(wpack)
    x16 = x.reshape(16, C, n).astype(f16)
    in_maps = []
    for i in range(8):
        xs, xt = x16[i], x16[i + 8]
        cs_s = xs.sum(axis=1, dtype=f32)
        cs_t = xt.sum(axis=1, dtype=f32)
        rpack = np.concatenate([
            qb, kb, kw @ cs_s + n * kb, kw @ cs_t + n * kb,
            qw @ cs_s, qw @ cs_t]).reshape(1, 6 * C).astype(f16)
        in_maps.append({
            "xs": xs,
            "xt": xt,
            "wpack": wpack,
            "rpack": np.ascontiguousarray(rpack),
        })
    return in_maps


_NC_CACHE = {}


def run_device(x, qw, qb, kw, kb, vw, vb, trace=False):
    from concourse.bass_utils import run_bass_kernel_spmd

    if "nc" not in _NC_CACHE:
        _NC_CACHE["nc"] = build_nc(N_FULL)
    nc = _NC_CACHE["nc"]
    in_maps = prep_core_inputs(x, qw, qb, kw, kb, vw, vb)
    res = run_bass_kernel_spmd(nc, in_maps, core_ids=list(range(8)),
                               trace=trace)
    y = np.empty((16, C, 128, 128), np.float32)
    for i in range(8):
        y[i] = res.results[i]["ys"].reshape(C, 128, 128)
        y[i + 8] = res.results[i]["yt"].reshape(C, 128, 128)
    return y, res


def kernel(**inputs):
    y, _ = run_device(
        np.asarray(inputs["x"]), np.asarray(inputs["qw"]),
        np.asarray(inputs["qb"]), np.asarray(inputs["kw"]),
        np.asarray(inputs["kb"]), np.asarray(inputs["vw"]),
        np.asarray(inputs["vb"]),
    )
    return y


# revision 26
# speedup vs baseline: 1.0138x; 1.0138x over previous
"""Trainium2 Bass kernel for nn_Cca3 channel cross-attention.

Reference computation (per pair b of 8):
  x_s, x_t : [128, N] (N = 128*128 spatial), C = 128 channels
  q/k/v = 1x1 conv projections (w @ x + b) of both streams
  S1 = q_t @ k_s^T  (contract over N) -> a_st = rowsoftmax(S1)
  S2 = q_s @ k_t^T                    -> a_ts = rowsoftmax(S2)
  att = rowsoftmax(a_st @ a_ts^T)
  out_s = x_s + att @ v_s ; out_t = x_t + att @ v_t

Sharding: data-parallel, one (x_s[i], x_t[i]) pair per NeuronCore (8 cores).

Device strategy (per core), fp16 operands / fp32 PSUM accumulation:
  - DMA economics: every dma_start costs ~0.7 us of issue time on the Sync
    engine and one queue runs at ~22 GB/s, so input is 16x2 slabs of 128 KB
    (512 cols) issued first, weights packed into two small tensors, and
    output rides 256 KB DMAs with a tapered tail.
  - Warmup burst: back-to-back 512-wide matmuls on a zero scratch flip the
    PE HAM clock gate to 2.4 GHz while the input DMA pipe fills (otherwise
    the short phase-1 matmuls never fill an activity window and the whole
    kernel runs at 1.2 GHz).
  - Scores via the Gram matrix: with projections P = w X (sans bias),
      S2_raw = qw (Xs Xt^T) kw^T = qw G kw^T,   S1_raw = qw G^T kw^T.
    Phase 1 only accumulates G [128,128]: per 128-col chunk, PE-transpose
    xs/xt chunks (fp16) into a ring; G matmuls trail by one 4-chunk batch
    (transposes batched so the identity stationary loads once per batch).
    Transposes of 2 chunks share one PSUM tile (PSUM is bank-granular);
    ring copies are whole-tile [128,512] DVE/ACT ops, alternating.
  - Bias corrections are rank-1: S1 += outer(qb, K0s + N kb) + outer(Q0t, kb)
    with the row vectors (projection column sums = w @ colsum(x)) computed on
    host from the same fp16 x the device sees, fed as tiny fp16 inputs.
  - Epilogue (all [128,128], fp16 operands): G -> S1/S2 via two matmuls
    each, softmax chains on DVE+ACT (exp with accum_out gives the row sum
    free), att composition via PE transposes + one 128^3 matmul.
  - Phase 2 folds v-projection, attention apply, and residual into ONE
    stationary weight:  y = (att vw + I) x + (att vb) (x) 1
    -> per 1024-col tile: two fp16 matmuls (constant stationary W'^T),
    two biased copies PSUM->SBUF fp16 (ACT/DVE alternating), one 256 KB
    output DMA; the last tiles shrink to keep the drain tail short.
"""

from contextlib import ExitStack

import numpy as np

C = 128
N_FULL = 16384
SLAB = 512  # input DMA slab width (fp16 -> 128 KB per slab)
F2 = 512  # phase-2 matmul/copy width (one PSUM bank)
TSLOT = 256  # ring slot: [xTs(128) | xTt(128)], fp16
TBUFS = 12  # ring depth (even, so chunk pairs land in adjacent slots)
TBATCH = 4  # transpose batch (identity stationary reloads once per batch)
NWARM = 16  # HAM warmup matmuls


def build_nc(n=N_FULL):
    import concourse.bacc as bacc
    import concourse.tile as tile
    from concourse import mybir
    from concourse.masks import make_identity

    f32 = mybir.dt.float32
    f16 = mybir.dt.float16
    AF = mybir.ActivationFunctionType
    AX = mybir.AxisListType

    slab = min(SLAB, n)
    nslabs = n // slab
    nchunks = n // C
    nbatches = nchunks // TBATCH
    assert TBUFS % 2 == 0 and TBUFS >= 2 * TBATCH

    nc = bacc.Bacc("TRN2", target_bir_lowering=False, debug=False)

    def din(name, shape, dt=f16):
        return nc.dram_tensor(name, shape, dt, kind="ExternalInput").ap()

    def dout(name, shape, dt=f16):
        return nc.dram_tensor(name, shape, dt, kind="ExternalOutput").ap()

    xs_d = din("xs", [C, n])
    xt_d = din("xt", [C, n])
    # packed weights: [qwT | kwT | vw | vb_col]
    wpack_d = din("wpack", [C, 3 * C + 1])
    # packed rows: [qb | kb | cks | ckt | q0s | q0t]
    rpack_d = din("rpack", [1, 6 * C])
    ys_d = dout("ys", [C, n])
    yt_d = dout("yt", [C, n])

    with tile.TileContext(nc) as tc, ExitStack() as ctx:
        singles = ctx.enter_context(tc.tile_pool(name="singles", bufs=1))

        # ---- persistent SBUF ----
        xs_sb = singles.tile([C, n], f16, tag="xs")
        xt_sb = singles.tile([C, n], f16, tag="xt")
        wpack_sb = singles.tile([C, 3 * C + 1], f16, tag="wpack")
        rpack_sb = singles.tile([1, 6 * C], f16, tag="rpack")
        ident16 = singles.tile([C, C], f16, tag="ident16")
        tring = singles.tile([C, TBUFS * TSLOT], f16, tag="tring")
        scratch = singles.tile([C, 512], f16, tag="scratch")
        warm_sb = singles.tile([1, 2], f32, tag="warm")

        qwT_sb = wpack_sb[:, 0:C]
        kwT_sb = wpack_sb[:, C : 2 * C]
        vw_sb = wpack_sb[:, 2 * C : 3 * C]
        vb_sb = wpack_sb[:, 3 * C : 3 * C + 1]
        qb_sb = rpack_sb[0:1, 0:C]
        kb_sb = rpack_sb[0:1, C : 2 * C]
        cks_sb = rpack_sb[0:1, 2 * C : 3 * C]
        ckt_sb = rpack_sb[0:1, 3 * C : 4 * C]
        q0s_sb = rpack_sb[0:1, 4 * C : 5 * C]
        q0t_sb = rpack_sb[0:1, 5 * C : 6 * C]

        # ---- input slabs first: Sync issue slots are the scarce resource
        for k in range(nslabs):
            sl = slice(k * slab, (k + 1) * slab)
            nc.sync.dma_start(out=xs_sb[:, sl], in_=xs_d[:, sl])
            nc.sync.dma_start(out=xt_sb[:, sl], in_=xt_d[:, sl])
            if k == 1:
                nc.sync.dma_start(out=wpack_sb, in_=wpack_d)
                nc.sync.dma_start(out=rpack_sb, in_=rpack_d)

        # HAM warmup: long back-to-back matmuls while the input pipe fills
        nc.vector.memset(scratch, 0.0)
        make_identity(nc, ident16)
        with tc.tile_pool(name="warmps", bufs=2, space="PSUM") as warm_ps:
            for _ in range(NWARM):
                wp = warm_ps.tile([C, 512], f32, tag="wp")
                nc.tensor.matmul(wp, lhsT=scratch[:, 0:C], rhs=scratch,
                                 start=True, stop=True)
        # warm the ACT exp table early (overlaps input DMA)
        nc.vector.memset(warm_sb, 0.0)
        nc.scalar.activation(out=warm_sb, in_=warm_sb, func=AF.Exp)

        # =========================== phase 1 ===========================
        smalls = ctx.enter_context(tc.tile_pool(name="smalls", bufs=1))
        g16_sb = smalls.tile([C, C], f16, tag="g16")
        gt16_sb = smalls.tile([C, C], f16, tag="gt16")
        m1_sb = smalls.tile([C, C], f16, tag="m1")
        m2_sb = smalls.tile([C, C], f16, tag="m2")
        ast_sb = smalls.tile([C, C], f16, tag="ast")
        ats_sb = smalls.tile([C, C], f16, tag="ats")
        att_sb = smalls.tile([C, C], f16, tag="att")
        astT_sb = smalls.tile([C, C], f16, tag="astT")
        atsT_sb = smalls.tile([C, C], f16, tag="atsT")
        attT_sb = smalls.tile([C, C], f16, tag="attT")
        wt_sb = smalls.tile([C, C], f16, tag="wt")  # (E_m vw + diag(s_m))^T
        identS_sb = smalls.tile([C, C], f16, tag="identS")  # diag(s_m)
        ceff2_sb = smalls.tile([C, 1], f32, tag="ceff2")  # (E_m vb) * rinv_m
        rinvm_sb = smalls.tile([C, 1], f32, tag="rinvm")  # 1 / s_m
        biasm_sb = smalls.tile([C, 1], f32, tag="biasm")

        with tc.tile_pool(name="gps", bufs=1, space="PSUM") as g_ps_pool:
            G = g_ps_pool.tile([C, C], f32, tag="G")

            with tc.tile_pool(name="trps", bufs=4, space="PSUM") as tr_ps:

                def emit_tr_pair(p):
                    # chunks 2p, 2p+1 -> one PSUM tile, one ring copy
                    i0 = 2 * p
                    st = (i0 % TBUFS) * TSLOT
                    psT = tr_ps.tile([C, 4 * C], f16, tag="psT")
                    for u in range(2):
                        sl = slice((i0 + u) * C, (i0 + u + 1) * C)
                        nc.tensor.transpose(psT[:, 2 * u * C : (2 * u + 1) * C],
                                            xs_sb[:, sl], ident16)
                        nc.tensor.transpose(
                            psT[:, (2 * u + 1) * C : (2 * u + 2) * C],
                            xt_sb[:, sl], ident16)
                    if p % 16 < 9:
                        nc.vector.tensor_copy(tring[:, st : st + 4 * C], psT)
                    else:
                        nc.scalar.copy(tring[:, st : st + 4 * C], psT)

                def emit_g_batch(b):
                    for j in range(b * TBATCH, (b + 1) * TBATCH):
                        st = (j % TBUFS) * TSLOT
                        nc.tensor.matmul(G, lhsT=tring[:, st : st + C],
                                         rhs=tring[:, st + C : st + 2 * C],
                                         start=(j == 0),
                                         stop=(j == nchunks - 1))

                pairs_per_batch = TBATCH // 2
                for b in range(nbatches + 1):
                    if b < nbatches:
                        for p in range(b * pairs_per_batch,
                                       (b + 1) * pairs_per_batch):
                            emit_tr_pair(p)
                    if b >= 1:
                        emit_g_batch(b - 1)

            nc.vector.tensor_copy(g16_sb, G)

        # ---- epilogue: G -> S1/S2 -> att -> W' (all fp16) ----
        def rowsoftmax(src, dst, tg):
            nmx = smalls.tile([C, 1], f32, tag=tg + "nmx")
            ssum = smalls.tile([C, 1], f32, tag=tg + "ssum")
            rinv = smalls.tile([C, 1], f32, tag=tg + "rinv")
            nc.vector.reduce_max(nmx, src, axis=AX.X, negate=True)
            nc.scalar.activation(out=dst, in_=src, func=AF.Exp,
                                 bias=nmx, scale=1.0, accum_out=ssum)
            nc.vector.reciprocal(rinv, ssum)
            nc.vector.tensor_scalar_mul(dst, dst, rinv)

        with tc.tile_pool(name="eps", bufs=2, space="PSUM") as e_ps, \
             tc.tile_pool(name="sps", bufs=1, space="PSUM") as s_ps, \
             tc.tile_pool(name="fillps", bufs=1, space="PSUM") as fill_ps:
            # independent scratch matmuls keep the PE activity monitor happy
            # through the epilogue's serial chain (else the clock gate drops
            # back to 1.2 GHz and phase 2 starts cold)
            fp = fill_ps.tile([C, 512], f32, tag="fp")

            def pe_fill(k=1):
                for _ in range(k):
                    nc.tensor.matmul(fp, lhsT=scratch[:, 0:C], rhs=scratch,
                                     start=True, stop=True)

            # S2 branch first (no dependence on the G transpose)
            m2_ps = e_ps.tile([C, C], f32, tag="e")
            nc.tensor.matmul(m2_ps, lhsT=g16_sb, rhs=qwT_sb,
                             start=True, stop=True)
            gt_ps = e_ps.tile([C, C], f16, tag="t")
            nc.tensor.transpose(gt_ps, g16_sb, ident16)
            nc.vector.tensor_copy(m2_sb, m2_ps)
            nc.scalar.copy(gt16_sb, gt_ps)
            pe_fill(1)
            S2 = s_ps.tile([C, C], f32, tag="S2")
            S1 = s_ps.tile([C, C], f32, tag="S1")
            nc.tensor.matmul(S2, lhsT=m2_sb, rhs=kwT_sb,
                             start=True, stop=True)
            nc.tensor.matmul(S2, lhsT=qb_sb, rhs=ckt_sb, start=False,
                             stop=False, skip_group_check=True)
            nc.tensor.matmul(S2, lhsT=q0s_sb, rhs=kb_sb, start=False,
                             stop=True, skip_group_check=True)
            m1_ps = e_ps.tile([C, C], f32, tag="e")
            nc.tensor.matmul(m1_ps, lhsT=gt16_sb, rhs=qwT_sb,
                             start=True, stop=True)
            nc.vector.tensor_copy(m1_sb, m1_ps)
            pe_fill(1)
            nc.tensor.matmul(S1, lhsT=m1_sb, rhs=kwT_sb,
                             start=True, stop=True)
            nc.tensor.matmul(S1, lhsT=qb_sb, rhs=cks_sb, start=False,
                             stop=False, skip_group_check=True)
            nc.tensor.matmul(S1, lhsT=q0t_sb, rhs=kb_sb, start=False,
                             stop=True, skip_group_check=True)

            pe_fill(2)
            # a_ts: full rowsoftmax (its row scale changes the att logits)
            rowsoftmax(S2, ats_sb, "s2")
            # a_st: UNnormalized exp; its 1/sum folds into the att-exp scale
            nmx1 = smalls.tile([C, 1], f32, tag="nmx1")
            ssum1 = smalls.tile([C, 1], f32, tag="ssum1")
            rinv1 = smalls.tile([C, 1], f32, tag="rinv1")
            nc.vector.reduce_max(nmx1, S1, axis=AX.X, negate=True)
            nc.scalar.activation(out=ast_sb, in_=S1, func=AF.Exp,
                                 bias=nmx1, scale=1.0, accum_out=ssum1)
            nc.vector.reciprocal(rinv1, ssum1)
            pe_fill(2)
            t2 = e_ps.tile([C, C], f16, tag="t")
            nc.tensor.transpose(t2, ats_sb, ident16)
            nc.scalar.copy(atsT_sb, t2)
            t1 = e_ps.tile([C, C], f16, tag="t")
            nc.tensor.transpose(t1, ast_sb, ident16)
            nc.vector.tensor_copy(astT_sb, t1)
            pe_fill(1)
            m_ps = e_ps.tile([C, C], f32, tag="e")
            nc.tensor.matmul(m_ps, lhsT=astT_sb, rhs=atsT_sb,
                             start=True, stop=True)
            pe_fill(2)
            # att = softmax(rinv1 * Z_u) row-wise, kept UNnormalized:
            # E_m = exp(Z_u*rinv1 - max*rinv1); 1/s_m folds into phase 2
            nmz = smalls.tile([C, 1], f32, tag="nmz")
            ssumm = smalls.tile([C, 1], f32, tag="ssumm")
            nc.vector.reduce_max(nmz, m_ps, axis=AX.X, negate=True)
            nc.vector.tensor_mul(biasm_sb, nmz, rinv1)
            nc.scalar.activation(out=att_sb, in_=m_ps, func=AF.Exp,
                                 bias=biasm_sb, scale=rinv1,
                                 accum_out=ssumm)
            nc.vector.reciprocal(rinvm_sb, ssumm)
            nc.vector.tensor_scalar_mul(identS_sb, ident16, ssumm)
            pe_fill(2)
            t3 = e_ps.tile([C, C], f16, tag="t")
            nc.tensor.transpose(t3, att_sb, ident16)
            nc.vector.tensor_copy(attT_sb, t3)
            pe_fill(1)
            # W''^T = (E_m vw)^T + diag(s_m); y = (W'' x + E_m vb) * rinv_m
            wt_ps = e_ps.tile([C, C], f32, tag="e")
            nc.tensor.matmul(wt_ps, lhsT=vw_sb, rhs=attT_sb,
                             start=True, stop=True)
            nc.vector.tensor_add(wt_sb, wt_ps, identS_sb)
            ce_ps = s_ps.tile([C, 1], f32, tag="ce")
            nc.tensor.matmul(ce_ps, lhsT=attT_sb, rhs=vb_sb,
                             start=True, stop=True)
            nc.vector.tensor_mul(ceff2_sb, ce_ps, rinvm_sb)

        # =========================== phase 2 ===========================
        # y = W' x + ceff (x) 1 ; W'^T constant stationary for all chunks.
        # One y tile = `parts` x F2 columns, one output DMA per tile;
        # tail tiles shrink so the final queue drain is short.
        # per stream: 14 x 1024-col tiles, 2 x 512, 2 x 256 (tapered tail so
        # the final per-queue drain is ~3 us instead of ~12)
        spans = [(j * 1024, 1024) for j in range(14)]
        spans += [(14336, 512), (14848, 512), (15360, 512),
                  (15872, 256), (16128, 256)]
        assert sum(w for _, w in spans) == n
        tiles = []  # (x_sb, y_d, col_start, width)
        for col, w in spans:
            tiles.append((xs_sb, ys_d, col, w))
            tiles.append((xt_sb, yt_d, col, w))

        with tc.tile_pool(name="ops", bufs=6, space="PSUM") as o_ps_pool, \
             tc.tile_pool(name="ysb", bufs=12) as y_sb_pool:
            ci = 0
            for idx, (x_sb, y_d, col, width) in enumerate(tiles):
                y_sb = y_sb_pool.tile([C, 2 * F2], f16, tag="y")
                for h in range(0, width, F2):
                    w = min(F2, width - h)
                    sl = slice(col + h, col + h + w)
                    o_ps = o_ps_pool.tile([C, F2], f32, tag="o")
                    nc.tensor.matmul(o_ps[:, 0:w], lhsT=wt_sb,
                                     rhs=x_sb[:, sl], start=True, stop=True)
                    dst = y_sb[:, h : h + w]
                    ci += 1
                    if ci % 2 == 0:
                        nc.vector.tensor_scalar(
                            dst, o_ps[:, 0:w], rinvm_sb, ceff2_sb,
                            mybir.AluOpType.mult, mybir.AluOpType.add)
                    else:
                        nc.scalar.activation(out=dst, in_=o_ps[:, 0:w],
                                             func=AF.Identity,
                                             bias=ceff2_sb, scale=rinvm_sb)
                # two issue engines: Sync and GpSimd (SWDGE) share the
                # per-dma ~0.7us descriptor-generation cost
                eng = nc.gpsimd if idx % 2 == 0 else nc.sync
                eng.dma_start(out=y_d[:, col : col + width],
                              in_=y_sb[:, 0:width])

    nc.compile()
    return nc


def prep_core_inputs(x, qw, qb, kw, kb, vw, vb, n=N_FULL):
    """Build the 8 per-core input maps from full inputs."""
    f32, f16 = np.float32, np.float16
    qw = qw.astype(f32)
    kw = kw.astype(f32)
    wpack = np.concatenate(
        [qw.T, kw.T, vw.astype(f32), vb.reshape(C, 1)], axis=1).astype(f16)
    wpack = np.ascontiguousarray(wpack)
    x16 = x.reshape(16, C, n).astype(f16)
    in_maps = []
    for i in range(8):
        xs, xt = x16[i], x16[i + 8]
        cs_s = xs.sum(axis=1, dtype=f32)
        cs_t = xt.sum(axis=1, dtype=f32)
        rpack = np.concatenate([
            qb, kb, kw @ cs_s + n * kb, kw @ cs_t + n * kb,
            qw @ cs_s, qw @ cs_t]).reshape(1, 6 * C).astype(f16)
        in_maps.append({
            "xs": xs,
            "xt": xt,
            "wpack": wpack,
            "rpack": np.ascontiguousarray(rpack),
        })
    return in_maps


_NC_CACHE = {}


def run_device(x, qw, qb, kw, kb, vw, vb, trace=False):
    from concourse.bass_utils import run_bass_kernel_spmd

    if "nc" not in _NC_CACHE:
        _NC_CACHE["nc"] = build_nc(N_FULL)
    nc = _NC_CACHE["nc"]
    in_maps = prep_core_inputs(x, qw, qb, kw, kb, vw, vb)
    res = run_bass_kernel_spmd(nc, in_maps, core_ids=list(range(8)),
                               trace=trace)
    y = np.empty((16, C, 128, 128), np.float32)
    for i in range(8):
        y[i] = res.results[i]["ys"].reshape(C, 128, 128)
        y[i + 8] = res.results[i]["yt"].reshape(C, 128, 128)
    return y, res


def kernel(**inputs):
    y, _ = run_device(
        np.asarray(inputs["x"]), np.asarray(inputs["qw"]),
        np.asarray(inputs["qb"]), np.asarray(inputs["kw"]),
        np.asarray(inputs["kb"]), np.asarray(inputs["vw"]),
        np.asarray(inputs["vb"]),
    )
    return y
